# revision 1
# baseline (speedup 1.0000x reference)
"""UAVid mask conversion on 8 Trainium2 NeuronCores.

Input:  tensor [3, 2160, 3840] int32 (CHW RGB image)
Output: [2160, 3840] uint8 class ids (8-entry palette exact match, else 0)

Per core (H sharded 8-ways => [3, 270, 3840]):
  1. DMA int32 channel planes to SBUF (sync HWDGE queue, one 3D-AP DMA
     per chunk). Graduated chunk sizes: small at both ends for fast
     pipeline fill/drain, 1080-wide in the middle for low overhead.
  2. ScalarE converts int32 -> bf16 (values 0..255, exact).
  3. TensorE builds key = r*65536 + g*256 + b via three diagonal-matrix
     matmuls accumulated in PSUM fp32 (exact: key < 2^24).
  4. VectorE runs a 3-op custom-DVE replace cascade mapping the 7 nonzero
     palette keys to -class, then (t<0)*(0-t) -> uint8.
     Key identities pack 3 compare/select entries in one op:
       k5 = k1 + k4, and k2 = k6 + k7.
  5. DMA uint8 out (gpsimd SWDGE mid-stream, sync for the last two).
"""

import sys

if "/opt/trn_rl_repo" not in sys.path:
    sys.path.insert(0, "/opt/trn_rl_repo")

import numpy as np

H, W = 2160, 3840
NCORES = 8
HSH = H // NCORES            # 270 rows per core
NPIX = HSH * W               # 1036800 pixels per core
P = 128                      # SBUF partitions
FD = NPIX // P               # 8100 elements per partition
CH = 1080                    # max free-dim chunk (tile width)
CHUNKS = [270, 270, 540, 540, 1080, 1080, 1080, 1080, 1080, 810, 270]
NCH = len(CHUNKS)
NFILL = 3                    # leading chunks: key on DVE (no act/mm chain)

assert P * FD == NPIX and sum(CHUNKS) == FD

# palette keys (r<<16 | g<<8 | b)
K1 = 8388608.0   # (128,0,0)    -> 1
K2 = 8405120.0   # (128,64,128) -> 2
K3 = 12583104.0  # (192,0,192)  -> 3
K4 = 32768.0     # (0,128,0)    -> 4
K5 = 8421376.0   # (128,128,0)  -> 5  (= K1 + K4)
K6 = 4210688.0   # (64,64,0)    -> 6
K7 = 4194432.0   # (64,0,128)   -> 7  (K6 + K7 = K2)

_OPS = None      # (OP_B, OP_C, OP_D)
_PROG = None     # compiled Bass program
_WKEY = None     # host-side bf16 weight constant


def _register_custom_ops():
    """Build + register the three cascade DveOps in dve_ops.OPS."""
    global _OPS
    if _OPS is not None:
        return _OPS
    from concourse import dve_ops
    from concourse.dve_ops import DveOp, OPS, CUSTOM_DVE_SPECS
    from concourse.dve_spec import (
        Spec, Src0, Src1, C0, C1, C2, C3, Zero, One,
        select, eq, lower, AluOp, Bin, _spill_c3_to_src1,
    )
    from concourse.dve_uop import DveOpSpec

    def SUB(a, b): return Bin(AluOp.SUBTRACT, a, b)
    def ADD(a, b): return Bin(AluOp.ADD, a, b)
    def MUL(a, b): return Bin(AluOp.MULTIPLY, a, b)
    def LT(a, b): return Bin(AluOp.IS_LT, a, b)

    f = np.float32

    def _scal(c):
        a = np.asarray(c, np.float32)
        return a if a.ndim else f(float(a))

    # OP_B: replace k1 -> -1 (latched 0-1), k4 -> imm2, k1+k4 -> in1-spill
    t1 = select(eq(Src0, C0), SUB(Zero, One), Src0)
    t2 = select(eq(t1, C1), C2, t1)
    t3 = select(eq(t2, ADD(C0, C1)), C3, t2)

    def _ref_b(in0, in1, c0, c1, c2):
        v = np.asarray(in0, f)
        s0, s1 = _scal(c0), _scal(c1)
        sp = np.asarray(in1, f).reshape(-1, 1)
        v = np.where(v == s0, f(-1.0), v).astype(f)
        v = np.where(v == s1, f(c2), v).astype(f)
        v = np.where(v == np.asarray(s0 + s1, f),
                     np.broadcast_to(sp, v.shape), v).astype(f)
        return v

    SPEC_B = Spec(body=_spill_c3_to_src1(t3), reference=_ref_b)

    # OP_C / OP_D shared shape: replace s0 -> imm2, s1 -> in1-spill
    def _two_entry_body():
        u1 = select(eq(Src0, C0), C2, Src0)
        return select(eq(u1, C1), C3, u1)

    def _ref_c(in0, in1, c0, c1, c2):
        v = np.asarray(in0, f)
        s0, s1 = _scal(c0), _scal(c1)
        sp = np.asarray(in1, f).reshape(-1, 1)
        v = np.where(v == s0, f(c2), v).astype(f)
        v = np.where(v == s1, np.broadcast_to(sp, v.shape), v).astype(f)
        return v

    SPEC_C = Spec(body=_spill_c3_to_src1(_two_entry_body()), reference=_ref_c)

    # OP_D: two entries + extract: out = (t < 0) * (0 - t)
    wd = _two_entry_body()
    body_d = _spill_c3_to_src1(MUL(LT(wd, Zero), SUB(Zero, wd)))

    def _ref_d(in0, in1, c0, c1, c2):
        v = _ref_c(in0, in1, c0, c1, c2)
        return (np.float32(v < 0) * (f(0.0) - v)).astype(f)

    SPEC_D = Spec(body=body_d, reference=_ref_d)

    built = []
    for name, spec in [
        ("UAVID_CAS3_ANT", SPEC_B),
        ("UAVID_CAS2_ANT", SPEC_C),
        ("UAVID_FIN2_ANT", SPEC_D),
    ]:
        if name in dve_ops._SUB_OPCODE_FOR_NAME:
            built.append(next(o for o in OPS if o.name == name))
            continue
        opcode = dve_ops._CUSTOM_DVE_ROW_BASE + len(OPS)
        assert opcode < 0x20, "custom DVE opcode rows exhausted"
        shas = {}
        for ver in ("v3", "v4"):
            tmp = DveOpSpec(name=name, opcode=opcode,
                            uops=lower(spec, ver=ver), rd1_en=True)
            shas[ver] = tmp.sha(ver)
        op = DveOp(name, spec, subdim=False, uops_sha=shas)
        OPS.append(op)
        dve_ops._SUB_OPCODE_FOR_NAME[name] = opcode
        CUSTOM_DVE_SPECS[name] = spec
        built.append(op)

    _OPS = tuple(built)
    return _OPS


_TAIL_PATCHED = False


def _patch_cheap_tail():
    """Replace TileContext._drain_and_barrier's second full butterfly with a
    sequencer-level (sem_only) barrier — the drain + first barrier already
    quiesced engines; the second barrier only orders the semaphore clears
    before NEFF end."""
    global _TAIL_PATCHED
    if _TAIL_PATCHED:
        return
    from concourse.tile import TileContext
    from concourse.vector_clock import ScopedClock

    def _cheap(self, tick_clock, wait_clock):
        drain_inst = self.nc.sync.drain()
        wait_clock.add_sem_waits(
            drain_inst.ins, ScopedClock({None: tick_clock.global_clock})
        )
        self.nc.all_engine_barrier()
        assert self.sems is not None
        popped = self.nc._tile_sem_poison_stack.pop()
        assert popped is self._sem_poison
        self.nc.clear_and_free_semaphores(list(self.sems.allocated().values()))
        self.nc.all_engine_barrier(sem_only=True)

    TileContext._drain_and_barrier = _cheap
    _TAIL_PATCHED = True


def _wkey_host():
    """[128, 384] bf16: concat of diag(65536), diag(256), diag(1)."""
    global _WKEY
    if _WKEY is None:
        import ml_dtypes
        w = np.zeros((P, 3 * P), np.float32)
        for i in range(P):
            w[i, i] = 65536.0
            w[i, P + i] = 256.0
            w[i, 2 * P + i] = 1.0
        _WKEY = w.astype(ml_dtypes.bfloat16)
    return _WKEY


def _build_program():
    """Trace + compile the single-core Bass/Tile program (SPMD on 8 cores)."""
    global _PROG
    if _PROG is not None:
        return _PROG

    from concourse import bacc, mybir
    from concourse.tile import TileContext

    _patch_cheap_tail()
    op_b, op_c, op_d = _register_custom_ops()

    nc = bacc.Bacc("TRN2", target_bir_lowering=False, debug=False)
    t_in = nc.dram_tensor("tensor", [3, HSH, W], mybir.dt.int32,
                          kind="ExternalInput").ap()
    t_wk = nc.dram_tensor("wkey", [P, 3 * P], mybir.dt.bfloat16,
                          kind="ExternalInput").ap()
    t_out = nc.dram_tensor("out", [HSH, W], mybir.dt.uint8,
                           kind="ExternalOutput").ap()

    in_pf = t_in.rearrange("c h w -> c (h w)").rearrange("c (p f) -> c p f", p=P)
    out_pf = t_out.rearrange("h w -> (h w)").rearrange("(p f) -> p f", p=P)

    copy_f = mybir.ActivationFunctionType.Copy

    with TileContext(nc) as tc:
        with tc.tile_pool(name="consts", bufs=1) as cpool:
            # constants off the HWDGE queues (gpsimd SWDGE + memsets)
            wk = cpool.tile([P, 3 * P], mybir.dt.bfloat16, tag="wk")
            nc.gpsimd.dma_start(out=wk[:], in_=t_wk[:])
            cm5 = cpool.tile([P, 1], mybir.dt.float32, tag="cm5")
            cm6 = cpool.tile([P, 1], mybir.dt.float32, tag="cm6")
            cm7 = cpool.tile([P, 1], mybir.dt.float32, tag="cm7")
            nc.gpsimd.memset(cm5[:], -5.0)
            nc.gpsimd.memset(cm6[:], -6.0)
            nc.gpsimd.memset(cm7[:], -7.0)

            with tc.tile_pool(name="psum", bufs=2, space="PSUM") as ppool, \
                 tc.tile_pool(name="sbuf", bufs=6) as pool:
                off = 0
                t_ms = 0.0
                for j, ch in enumerate(CHUNKS):
                    sl = slice(off, off + ch)
                    off += ch
                    # pace the scheduler's simulated readiness to the real
                    # DMA-stream arrival times so the static engine-stream
                    # order stays chunk-monotone (no late-chunk waits
                    # serializing earlier chunks' tails)
                    tc.tile_set_cur_wait(t_ms)
                    t_ms += 128 * 3 * ch * 4 / 380e9 * 1e3
                    # sync carries ONLY input DMAs: its issue stream never
                    # waits on compute, so transfers stream back-to-back.
                    # One 3D-AP DMA moves all three plane-chunks.
                    tin = pool.tile([P, 3 * CH], mybir.dt.int32, tag="tin")
                    tin3 = tin[:, 0:3 * ch].rearrange("p (c f) -> p c f", c=3)
                    nc.sync.dma_start(out=tin3, in_=in_pf[:, :, sl].rearrange(
                        "c p f -> p c f"))
                    tk = pool.tile([P, CH], mybir.dt.float32, tag="tk")
                    to = pool.tile([P, CH], mybir.dt.uint8, tag="to")
                    if j < NFILL:
                        # fill phase: key on DVE directly from int32 — the
                        # DVE is idle waiting for the pipeline anyway, and
                        # this skips the DMA->act->matmul chain latency
                        tu = pool.tile([P, CH], mybir.dt.float32, tag="tu")
                        nc.vector.scalar_tensor_tensor(
                            out=tu[:, 0:ch], in0=tin[:, ch:2 * ch],
                            scalar=256.0, in1=tin[:, 2 * ch:3 * ch],
                            op0=mybir.AluOpType.mult, op1=mybir.AluOpType.add)
                        nc.vector.scalar_tensor_tensor(
                            out=tk[:, 0:ch], in0=tin[:, 0:ch],
                            scalar=65536.0, in1=tu[:, 0:ch],
                            op0=mybir.AluOpType.mult, op1=mybir.AluOpType.add)
                        key_src = tk
                    else:
                        # int32 -> bf16 convert + key matmuls by 512-col
                        # group (matmuls start while later groups convert)
                        t16 = pool.tile([P, 3 * CH], mybir.dt.bfloat16,
                                        tag="t16")
                        pk = ppool.tile([P, CH], mybir.dt.float32, tag="pk")
                        tin3v = tin[:, 0:3 * ch].rearrange(
                            "p (c f) -> p c f", c=3)
                        t16v = t16[:, 0:3 * ch].rearrange(
                            "p (c f) -> p c f", c=3)
                        for s in range(0, ch, 512):
                            n = min(512, ch - s)
                            ssl = slice(s, s + n)
                            nc.scalar.activation(t16v[:, :, ssl],
                                                 tin3v[:, :, ssl], copy_f)
                            for pl, (w0, w1) in enumerate(((0, P), (P, 2 * P),
                                                           (2 * P, 3 * P))):
                                nc.tensor.matmul(
                                    pk[:, ssl], wk[:, w0:w1],
                                    t16[:, pl * ch + s:pl * ch + s + n],
                                    start=(pl == 0), stop=(pl == 2))
                        key_src = pk
                    # replace cascade on VectorE
                    nc.vector._custom_dve(op_b, out=tk[:, 0:ch],
                                          in0=key_src[:, 0:ch],
                                          in1=cm5[:], s0=K1, s1=K4, imm2=-4.0)
                    nc.vector._custom_dve(op_c, out=tk[:, 0:ch], in0=tk[:, 0:ch],
                                          in1=cm6[:], s0=K2, s1=K6, imm2=-2.0)
                    nc.vector._custom_dve(op_d, out=to[:, 0:ch], in0=tk[:, 0:ch],
                                          in1=cm7[:], s0=K3, s1=K7, imm2=-3.0)
                    # outputs on gpsimd SWDGE (sync stays input-only);
                    # the last two (issued after all input issues) on sync.
                    qo = nc.gpsimd if j < NCH - 2 else nc.sync
                    qo.dma_start(out=out_pf[:, sl], in_=to[:, 0:ch])

    nc.compile()
    _PROG = nc
    return nc


def _run(in_maps, trace=False, **kw):
    from concourse.bass_utils import run_bass_kernel_spmd
    nc = _build_program()
    return run_bass_kernel_spmd(nc, in_maps, core_ids=list(range(NCORES)),
                                trace=trace, **kw)


def make_in_maps(tensor):
    tensor = np.asarray(tensor)
    assert tensor.shape == (3, H, W), tensor.shape
    wk = _wkey_host()
    return [
        {"tensor": np.ascontiguousarray(tensor[:, i * HSH:(i + 1) * HSH, :],
                                        dtype=np.int32),
         "wkey": wk}
        for i in range(NCORES)
    ]


def kernel(tensor):
    res = _run(make_in_maps(tensor))
    outs = [np.asarray(res.results[i]["out"]).reshape(HSH, W)
            for i in range(NCORES)]
    return np.concatenate(outs, axis=0).astype(np.uint8)

